# revision 26
# baseline (speedup 1.0000x reference)
"""Trainium2 Bass kernel for nn_AttentionHead: causal attention head.

reference:
    scores = (E @ qk) @ E.T           # [N, N],  E: [4096, 2048]
    scores += causal_mask (strict upper = -inf)
    attn = softmax(scores, axis=-1)
    out = (attn @ E) @ ov             # [4096, 2048]

Strategy (8 NeuronCores, SPMD, no collectives):
  - Each core owns 4 query tiles of 128 rows, one per causal "band":
    core i owns global q-tiles {C*(B-1-t)+i : t in 0..B-1}, with key extents
    {128*C*(B-t)} = {4096, 3072, 2048, 1024}. Identical work on every core ->
    a single uniform instruction graph; only input DATA differs per core.
  - Score path needs ~16-bit precision (softmax scores are O(1000)). Instead
    of a 3-matmul fp16 hi/lo split, use 1 fp16 matmul (hi*hi) plus fp8
    DoubleRow matmuls for the dominant cross term (lo*hi), packing TWO
    128-deep contraction chunks per DR matmul (its two k-planes). The
    hi*lo key-side term is dropped: measured score err rms 0.35 -> final
    rel err 6e-3, still 3x under the 2e-2 gate. fp8 e4m3 of the lo parts
    is pre-scaled by 2^8 to stay in the normal range; the fp16 hi stationary
    operand is scaled by 2^8 to match, so one PSUM bank accumulates 256*S
    and the exp activation descales via scale=1/256. Same trick for
    Q = E @ qk.
  - Value path (attn @ E @ ov) runs in plain fp16 (softmax weights in [0,1]).
  - Softmax rows live on partitions ([q, j] layout): reduce_max / exp-with-
    bias / accum_out are all native per-partition ops. P tiles are then
    PE-transposed (128x128) so the PV matmul can contract over j.
  - Host prep is layout/dtype only: fp16/fp8 splits, transposes, tiling.

Dataflow per core (D=2048, DP=16 d-tiles, DPP=8 d-pairs, JW=512):
  Q^T[d',q]  = sum_d qk[d,d'] * EownT[d,q]         (384 MMs: 256 fp16 + 128 DR fp8)
  S[q,j]     = sum_d' Q^T[d',q] * ET[d',j]         (480 MMs: 320 fp16 + 160 DR fp8)
  P = exp((S256 - rowmax256)/256)                  (ACT, fp16 out, rowsum via accum_out)
  P^T tiles via PE transpose                       (80 transposes)
  OpT[d,q]   = sum_j E[j,d] * P^T[j,q]             (512 MMs fp16)
  out[q,d2]  = (sum_d OpT[d,q] * ov[d,d2]) / rowsum  (256 MMs fp16, N=512)
"""
import sys

for _p in ('/opt/trn_rl_repo', '/opt/pypackages'):
    if _p not in sys.path:
        sys.path.insert(0, _p)

import numpy as np
import ml_dtypes

# ---- configuration (hardcoded for the graded problem) ----
N_CTX = 4096
D_MODEL = 2048
N_CORES = 8
B_BANDS = 4
QT = 128                       # q-tile rows

MASK_NEG = -1e30
LSCALE = 256.0                 # 2^8 scale on fp16/fp8 low parts
F8NP = ml_dtypes.float8_e4m3   # TRN FP8_EXP4: max finite 240, inf beyond


def _f8(x):
    return np.clip(x, -240.0, 240.0).astype(F8NP)


def build_program(C=N_CORES, B=B_BANDS, D=D_MODEL):
    import concourse.bass as bass
    import concourse.mybir as mybir
    from concourse import bacc, tile
    from concourse.masks import make_identity

    F32 = mybir.dt.float32
    F16 = mybir.dt.float16
    F8 = mybir.dt.float8e4
    DR = mybir.MatmulPerfMode.DoubleRow

    N = C * B * QT                 # total context
    NQ = B * QT                    # rows per core
    DP = D // 128                  # d tiles
    DPP = DP // 2                  # d-tile pairs (DR fp8 packs 2 per MM)
    JW = min(512, QT * C)          # j / free-dim window
    NW = N // JW                   # S windows over full context
    NJT = N // 128                 # j tiles
    MASKW = QT * C                 # mask window width (last cols of each extent)
    NDC = D // JW                  # output d2 chunks

    exts = [QT * C * (B - t) for t in range(B)]   # extent per local q-tile t

    def n_jt(jt):                  # active moving width at j-tile jt
        return 128 * (B - jt // C)

    nc = bacc.Bacc("TRN2", target_bir_lowering=False, debug=False)

    # inputs (pre-tiled on host for contiguous DMA)
    qrth_d = nc.dram_tensor("qrth", [128, DP, NQ], F16, kind="ExternalInput")
    qrt8_d = nc.dram_tensor("qrt8", [128, DPP, 2, NQ], F8, kind="ExternalInput")
    wqh_d = nc.dram_tensor("wqh", [DP, 128, DP, 128], F16, kind="ExternalInput")
    wq8_d = nc.dram_tensor("wq8", [DP, 128, DPP, 2, 128], F8, kind="ExternalInput")
    eth_d = nc.dram_tensor("eth", [NW, 128, DP, JW], F16, kind="ExternalInput")
    e8_d = nc.dram_tensor("e8", [NW, 128, DPP, 2, JW], F8, kind="ExternalInput")
    ev_d = nc.dram_tensor("ev", [DP, 128, NJT, 128], F16, kind="ExternalInput")
    ov_d = nc.dram_tensor("ov", [NDC, 128, DP, JW], F16, kind="ExternalInput")
    mask_d = nc.dram_tensor("mask", [128, MASKW], F32, kind="ExternalInput")
    out_d = nc.dram_tensor("out", [NQ, D], F32, kind="ExternalOutput")

    with tile.TileContext(nc) as tc:
        with (
            tc.tile_pool(name="const", bufs=1) as constp,
            tc.tile_pool(name="qt", bufs=1) as qtp,
            tc.tile_pool(name="pt", bufs=1) as ptp,
            tc.tile_pool(name="small", bufs=1) as smallp,
            tc.tile_pool(name="mm_ps", bufs=4, space="PSUM") as mmps,
            tc.tile_pool(name="tr_ps", bufs=2, space="PSUM") as trps,
            tc.tile_pool(name="pv_ps", bufs=2, space="PSUM") as pvps,
        ):
            ident = constp.tile([128, 128], F16, tag="ident")
            make_identity(nc, ident[:])
            # mask load is issued later (after the Q-phase prologue DMAs) to
            # keep the first matmul's deps at the head of the DMA queues
            mask_sb = constp.tile([128, MASKW], F32, tag="mask")

            # PT[jt]: transposed attention weights, [j-part, q-cols prefix]
            pt = [ptp.tile([128, n_jt(jt)], F16, tag=f"pt{jt}", name=f"pt{jt}") for jt in range(NJT)]

            # S-phase stationary: qth = fp16(256*Q^T); q8[a] packs the fp8
            # low parts (ql256) of d-tiles 2a and 2a+1 as its two DR planes
            qth = [qtp.tile([128, NQ], F16, tag=f"qth{dp}", name=f"qth{dp}") for dp in range(DP)]
            q8 = [qtp.tile([128, 2, NQ], F8, tag=f"q8_{a}", name=f"q8_{a}") for a in range(DPP)]

            negmax = [smallp.tile([128, 1], F32, tag=f"ngm{t}", name=f"ngm{t}") for t in range(B)]
            rsum = [smallp.tile([128, 1], F32, tag=f"rs{t}", name=f"rs{t}") for t in range(B)]
            recip = [smallp.tile([128, 1], F32, tag=f"rc{t}", name=f"rc{t}") for t in range(B)]
            rspart = {}
            rmax = [smallp.tile([128, NW], F32, tag=f"rmx{t}", name=f"rmx{t}")
                    for t in range(B)]

            # ev pool opens before ew (proper nesting: released after PV)
            # so value tiles can prefetch during the S tail on the scalar
            # DMA queue, not head-blocking ew loads
            evp_cm = tc.tile_pool(name="evs", bufs=5)
            evp = evp_cm.__enter__()
            ev_tiles = {}

            def load_ev(dt):
                evs = evp.tile([128, NJT, 128], F16, tag="evs", name="evs")
                nc.scalar.dma_start(evs[:], ev_d[dt])
                ev_tiles[dt] = evs

            # ew pool lives across Q+S so window 0 can prefetch during Q
            ewp_cm = tc.tile_pool(name="ew", bufs=2)
            ewp = ewp_cm.__enter__()
            ew_tiles = {}

            def load_window(w):
                ewh = ewp.tile([128, DP, JW], F16, tag="ewh", name="ewh")
                ew8 = ewp.tile([128, DPP, 2, JW], F8, tag="ew8", name="ew8")
                nc.sync.dma_start(ewh[:], eth_d[w])
                nc.sync.dma_start(ew8[:], e8_d[w])
                ew_tiles[w] = (ewh, ew8)

            # ---------------- Phase Q: 256*Q^T = (256*qk)^T-contracted rows
            with (
                tc.tile_pool(name="qrt", bufs=1) as qrtp,
                tc.tile_pool(name="wq", bufs=3) as wqp,
                tc.tile_pool(name="qtmp", bufs=2) as qtmpp,
            ):
                wq_tiles = {}

                def load_wq(dp):
                    wqh_sl = wqp.tile([128, DP, 128], F16, tag="wqh", name="wqh")
                    wq8_sl = wqp.tile([128, DPP, 2, 128], F8, tag="wq8", name="wq8")
                    nc.sync.dma_start(wqh_sl[:], wqh_d[dp])
                    nc.sync.dma_start(wq8_sl[:], wq8_d[dp])
                    wq_tiles[dp] = (wqh_sl, wq8_sl)

                qrt_h = qrtp.tile([128, DP, NQ], F16, tag="qrh", name="qrh")
                qrt_8 = qrtp.tile([128, DPP, 2, NQ], F8, tag="qr8", name="qr8")
                # DMA triggers cost ~0.7us each on the sync engine and the
                # first ~6 get dedicated semaphores, so the prologue uses few,
                # fat, partition-major transfers ordered by first use: wq0's
                # first half + early qrt chunks gate the first matmuls.

                # stage the first matmuls' deps so the PE starts early
                wqh_sl0 = wqp.tile([128, DP, 128], F16, tag="wqh", name="wqh")
                wq8_sl0 = wqp.tile([128, DPP, 2, 128], F8, tag="wq8", name="wq8")
                wq_tiles[0] = (wqh_sl0, wq8_sl0)
                # qrt stream on the (idle) scalar engine's DMA queue, wq
                # stream on sync: triggers issue in parallel, no FIFO
                # head-blocking between the two streams
                nc.sync.dma_start(wqh_sl0[:, 0:DP // 2, :],
                                  wqh_d[0][:, 0:DP // 2, :])
                nc.scalar.dma_start(qrt_h[:, 0:4, :], qrth_d[:, 0:4, :])
                nc.scalar.dma_start(qrt_h[:, 4:8, :], qrth_d[:, 4:8, :])
                nc.sync.dma_start(wqh_sl0[:, DP // 2:, :],
                                  wqh_d[0][:, DP // 2:, :])
                nc.scalar.dma_start(qrt_h[:, 8:12, :], qrth_d[:, 8:12, :])
                nc.scalar.dma_start(qrt_h[:, 12:DP, :], qrth_d[:, 12:DP, :])
                nc.sync.dma_start(wq8_sl0[:], wq8_d[0])
                nc.scalar.dma_start(qrt_8[:, 0:DPP // 2], qrt8_d[:, 0:DPP // 2])
                nc.scalar.dma_start(qrt_8[:, DPP // 2:], qrt8_d[:, DPP // 2:])
                load_wq(1)
                load_wq(2)
                nc.sync.dma_start(mask_sb[:], mask_d[:])

                for dp in range(DP):
                    wqh_sl, wq8_sl = wq_tiles.pop(dp)
                    ps = mmps.tile([128, NQ], F32, tag="mm")
                    for dk in range(DP):
                        nc.tensor.matmul(ps[:], wqh_sl[:, dk], qrt_h[:, dk, :],
                                         start=(dk == 0), stop=False)
                    for a in range(DPP):
                        nc.tensor.matmul(ps[:], wq8_sl[:, a], qrt_8[:, a],
                                         start=False, stop=(a == DPP - 1),
                                         perf_mode=DR)
                    if dp + 3 < DP:
                        load_wq(dp + 3)
                    if dp == 2:
                        load_window(0)
                    nc.vector.tensor_copy(qth[dp][:], ps[:])
                    qh32 = qtmpp.tile([128, NQ], F32, tag="qh32")
                    nc.vector.tensor_copy(qh32[:], qth[dp][:])
                    # low fp8 plane: 256*Q - fp16(256*Q), already at 2^8 scale
                    nc.vector.tensor_sub(q8[dp // 2][:, dp % 2, :], ps[:], qh32[:])

            # ---------------- Phase S: scores + softmax + P^T
            with (
                tc.tile_pool(name="s", bufs=1) as sp,
                tc.tile_pool(name="p", bufs=2) as pp,
            ):
                s_t = [sp.tile([128, exts[t]], F32, tag=f"s{t}", name=f"s{t}") for t in range(B)]

                def softmax_t(t):
                    ext = exts[t]
                    nc.vector.reduce_max(
                        out=negmax[t][:], in_=rmax[t][:, :ext // JW],
                        axis=mybir.AxisListType.X, negate=True)
                    # descale: bias for exp must be -rowmax = -rowmax256/256
                    nc.vector.tensor_scalar_mul(
                        negmax[t][:], negmax[t][:], 1.0 / LSCALE)
                    for w2 in range(ext // JW):
                        pwin = pp.tile([128, JW], F16, tag=f"p{t}", name=f"p{t}")
                        rp = smallp.tile([128, 1], F32, tag=f"rsp{t}_{w2}", name=f"rsp{t}_{w2}")
                        rspart[(t, w2)] = rp
                        nc.scalar.activation(
                            pwin[:], s_t[t][:, w2 * JW:(w2 + 1) * JW],
                            mybir.ActivationFunctionType.Exp,
                            bias=negmax[t][:], scale=1.0 / LSCALE, accum_out=rp[:])
                        for jj in range(JW // 128):
                            jt = w2 * (JW // 128) + jj
                            trp = trps.tile([128, 128], F16, tag="tr")
                            nc.tensor.transpose(
                                trp[:], pwin[:, jj * 128:(jj + 1) * 128], ident[:])
                            nc.vector.tensor_copy(
                                pt[jt][:, t * 128:(t + 1) * 128], trp[:])
                    # rowsum = sum of window partials; recip
                    nc.vector.tensor_copy(rsum[t][:], rspart[(t, 0)][:])
                    for w2 in range(1, ext // JW):
                        nc.vector.tensor_add(
                            rsum[t][:], rsum[t][:], rspart[(t, w2)][:])
                    nc.vector.reciprocal(recip[t][:], rsum[t][:])

                for w in range(NW):
                    if w + 1 < NW:
                        load_window(w + 1)
                    if w == NW - 2:
                        for _g in range(3):
                            load_ev(_g)
                    elif w == NW - 1:
                        load_ev(3)
                        load_ev(4)
                    ewh, ew8 = ew_tiles.pop(w)
                    for t in range(B):
                        if exts[t] <= JW * w:
                            continue
                        ps = mmps.tile([128, JW], F32, tag="mm")
                        for dp in range(DP):
                            nc.tensor.matmul(ps[:], qth[dp][:, t * 128:(t + 1) * 128],
                                             ewh[:, dp],
                                             start=(dp == 0), stop=False)
                        for a in range(DPP):
                            nc.tensor.matmul(ps[:], q8[a][:, :, t * 128:(t + 1) * 128],
                                             ew8[:, a],
                                             start=False, stop=(a == DPP - 1),
                                             perf_mode=DR)
                        # copy scores to SBUF, folding in the causal mask on
                        # the last MASKW columns; track per-window row max
                        nmw = MASKW // JW
                        wloc = exts[t] // JW - 1 - w   # windows from the end
                        if wloc < nmw:
                            moff = (nmw - 1 - wloc) * JW
                            nc.vector.tensor_add(
                                s_t[t][:, w * JW:(w + 1) * JW], ps[:],
                                mask_sb[:, moff:moff + JW])
                        else:
                            nc.vector.tensor_copy(
                                s_t[t][:, w * JW:(w + 1) * JW], ps[:])
                        nc.vector.reduce_max(
                            out=rmax[t][:, w:w + 1],
                            in_=s_t[t][:, w * JW:(w + 1) * JW],
                            axis=mybir.AxisListType.X)
                        if JW * (w + 1) == exts[t]:
                            softmax_t(t)

            ewp_cm.__exit__(None, None, None)

            # ---------------- Phase PV: OpT[d, q] = sum_j E[j,d] P^T[j,q]
            with (
                tc.tile_pool(name="opt", bufs=1) as optp,
                tc.tile_pool(name="ovs", bufs=2) as ovp,
                tc.tile_pool(name="osb", bufs=2) as osbp,
            ):
                ov_tiles = {}

                def load_ov(dc):
                    ovs = ovp.tile([128, DP, JW], F16, tag="ovs", name="ovs")
                    nc.sync.dma_start(ovs[:], ov_d[dc])
                    ov_tiles[dc] = ovs

                opt = [optp.tile([128, NQ], F16, tag=f"opt{dt}", name=f"opt{dt}") for dt in range(DP)]
                NPRE = min(5, DP)   # tiles 0..4 already prefetched in S tail
                for dt in range(DP):
                    evs = ev_tiles.pop(dt)
                    ps = pvps.tile([128, NQ], F32, tag="pv")
                    for jt in range(NJT):
                        nw_ = n_jt(jt)
                        nc.tensor.matmul(ps[:, :nw_], evs[:, jt], pt[jt][:, :nw_],
                                         start=(jt == 0), stop=(jt == NJT - 1))
                    if dt + NPRE < DP:
                        load_ev(dt + NPRE)
                    elif dt == max(0, DP - NPRE):
                        load_ov(0)
                    elif dt == max(1, DP - NPRE + 1):
                        load_ov(1)
                    nc.vector.tensor_copy(opt[dt][:], ps[:])

                # ---------------- Phase O: out = (OpT^T @ ov) * recip
                if True:
                    for dc in range(NDC):
                        if dc + 2 < NDC:
                            load_ov(dc + 2)
                        ovs = ov_tiles.pop(dc)
                        for t in range(B):
                            ps = mmps.tile([128, JW], F32, tag="mm")
                            for dt in range(DP):
                                nc.tensor.matmul(
                                    ps[:], opt[dt][:, t * 128:(t + 1) * 128],
                                    ovs[:, dt],
                                    start=(dt == 0), stop=(dt == DP - 1))
                            osb = osbp.tile([128, JW], F32, tag="osb")
                            nc.vector.tensor_scalar_mul(osb[:], ps[:], recip[t][:])
                            nc.sync.dma_start(
                                out_d[t * 128:(t + 1) * 128,
                                      dc * JW:(dc + 1) * JW], osb[:])

            evp_cm.__exit__(None, None, None)

    nc.compile()
    return nc


def make_in_maps(embedding, qk, ov, C=N_CORES, B=B_BANDS):
    """Host-side layout/dtype prep. Returns (in_maps, gtiles_per_core)."""
    N, D = embedding.shape
    DP = D // 128
    DPP = DP // 2
    JW = min(512, QT * C)
    NW = N // JW
    NJT = N // 128
    NQ = B * QT
    NDC = D // JW
    MASKW = QT * C

    E = np.ascontiguousarray(embedding.astype(np.float32))
    ET = np.ascontiguousarray(E.T)
    Eh = E.astype(np.float16)
    ETh = np.ascontiguousarray(ET.astype(np.float16))
    ETh8 = _f8(ETh.astype(np.float32))
    W = qk.astype(np.float32)
    Wh = W.astype(np.float16)
    Wl = W - Wh.astype(np.float32)
    # 256*Wh is exact in fp16 (|W| < 1 so |256*Wh| < 65504)
    Wh256 = (LSCALE * Wh.astype(np.float32)).astype(np.float16)
    Wl8 = _f8(LSCALE * Wl)
    OVh = ov.astype(np.float16)

    eth_t = np.ascontiguousarray(
        ETh.reshape(DP, 128, NW, JW).transpose(2, 1, 0, 3))
    # e8[w, b, a, i, j] = fp8(ETh)[(2a+i)*128+b, w*JW+j]  (DR d-tile pairs)
    e8_t = np.ascontiguousarray(
        ETh8.reshape(DPP, 2, 128, NW, JW).transpose(3, 2, 0, 1, 4))
    wqh_t = np.ascontiguousarray(
        Wh256.reshape(DP, 128, DP, 128).transpose(2, 1, 0, 3))
    # wq8[c, b, a, i, d] = fp8(256*Wl)[(2a+i)*128+b, c*128+d]
    wq8_t = np.ascontiguousarray(
        Wl8.reshape(DPP, 2, 128, DP, 128).transpose(3, 2, 0, 1, 4))
    ev_t = np.ascontiguousarray(
        Eh.reshape(NJT, 128, DP, 128).transpose(2, 1, 0, 3))
    ov_t = np.ascontiguousarray(
        OVh.reshape(DP, 128, NDC, JW).transpose(2, 1, 0, 3))

    r = np.arange(128)[:, None]
    m = np.arange(MASKW)[None, :]

    in_maps = []
    gtiles_all = []
    for i in range(C):
        gtiles = [C * (B - 1 - t) + i for t in range(B)]
        gtiles_all.append(gtiles)
        qrh = np.concatenate(
            [ETh[:, 128 * g:128 * (g + 1)] for g in gtiles], axis=1)
        qr8h = np.concatenate(
            [ETh8[:, 128 * g:128 * (g + 1)] for g in gtiles], axis=1)
        # partition-major for fat contiguous DMA runs:
        # qrth[b, dk, q] = ETh_own[dk*128+b, q]
        # qrt8[b, a, i, q] = fp8(ETh_own)[(2a+i)*128+b, q]
        qrth = qrh.reshape(DP, 128, NQ).transpose(1, 0, 2)
        qrt8 = qr8h.reshape(DPP, 2, 128, NQ).transpose(2, 0, 1, 3)
        mask = np.where(m <= 128 * i + r, 0.0, MASK_NEG).astype(np.float32)
        in_maps.append({
            "qrth": np.ascontiguousarray(qrth),
            "qrt8": np.ascontiguousarray(qrt8),
            "wqh": wqh_t, "wq8": wq8_t,
            "eth": eth_t, "e8": e8_t,
            "ev": ev_t, "ov": ov_t,
            "mask": mask,
        })
    return in_maps, gtiles_all


_CACHED = {}


def kernel(embedding, qk, ov):
    from concourse.bass_utils import run_bass_kernel_spmd

    key = "main"
    if key not in _CACHED:
        _CACHED[key] = build_program()
    nc = _CACHED[key]

    in_maps, gtiles_all = make_in_maps(embedding, qk, ov)
    res = run_bass_kernel_spmd(nc, in_maps, core_ids=list(range(N_CORES)))

    N, D = embedding.shape
    out = np.empty((N, D), dtype=np.float32)
    for i in range(N_CORES):
        o = res.results[i]["out"]
        for t, g in enumerate(gtiles_all[i]):
            out[128 * g:128 * (g + 1)] = o[128 * t:128 * (t + 1)]
    return out


# revision 31
# speedup vs baseline: 1.0100x; 1.0100x over previous
"""Trainium2 Bass kernel for nn_AttentionHead: causal attention head.

reference:
    scores = (E @ qk) @ E.T           # [N, N],  E: [4096, 2048]
    scores += causal_mask (strict upper = -inf)
    attn = softmax(scores, axis=-1)
    out = (attn @ E) @ ov             # [4096, 2048]

Strategy (8 NeuronCores, SPMD, no collectives):
  - Each core owns 4 query tiles of 128 rows, one per causal "band":
    core i owns global q-tiles {C*(B-1-t)+i : t in 0..B-1}, with key extents
    {128*C*(B-t)} = {4096, 3072, 2048, 1024}. Identical work on every core ->
    a single uniform instruction graph; only input DATA differs per core.
  - Score path needs ~16-bit precision (softmax scores are O(1000)). Instead
    of a 3-matmul fp16 hi/lo split, use 1 fp16 matmul (hi*hi) plus fp8
    DoubleRow matmuls for the dominant cross term (lo*hi), packing TWO
    128-deep contraction chunks per DR matmul (its two k-planes). The
    hi*lo key-side term is dropped: measured score err rms 0.35 -> final
    rel err 6e-3, still 3x under the 2e-2 gate. fp8 e4m3 of the lo parts
    is pre-scaled by 2^8 to stay in the normal range; the fp16 hi stationary
    operand is scaled by 2^8 to match, so one PSUM bank accumulates 256*S
    and the exp activation descales via scale=1/256. Same trick for
    Q = E @ qk.
  - Value path (attn @ E @ ov) runs in plain fp16 (softmax weights in [0,1]).
  - Softmax rows live on partitions ([q, j] layout): reduce_max / exp-with-
    bias / accum_out are all native per-partition ops. P tiles are then
    PE-transposed (128x128) so the PV matmul can contract over j.
  - Host prep is layout/dtype only: fp16/fp8 splits, transposes, tiling.

Dataflow per core (D=2048, DP=16 d-tiles, DPP=8 d-pairs, JW=512):
  Q^T[d',q]  = sum_d qk[d,d'] * EownT[d,q]         (384 MMs: 256 fp16 + 128 DR fp8)
  S[q,j]     = sum_d' Q^T[d',q] * ET[d',j]         (480 MMs: 320 fp16 + 160 DR fp8)
  P = exp((S256 - rowmax256)/256)                  (ACT, fp16 out, rowsum via accum_out)
  P^T tiles via PE transpose                       (80 transposes)
  OpT[d,q]   = sum_j E[j,d] * P^T[j,q]             (512 MMs fp16)
  out[q,d2]  = (sum_d OpT[d,q] * ov[d,d2]) / rowsum  (256 MMs fp16, N=512)
"""
import sys

for _p in ('/opt/trn_rl_repo', '/opt/pypackages'):
    if _p not in sys.path:
        sys.path.insert(0, _p)

import numpy as np
import ml_dtypes

# ---- configuration (hardcoded for the graded problem) ----
N_CTX = 4096
D_MODEL = 2048
N_CORES = 8
B_BANDS = 4
QT = 128                       # q-tile rows

MASK_NEG = -1e30
LSCALE = 256.0                 # 2^8 scale on fp16/fp8 low parts
F8NP = ml_dtypes.float8_e4m3   # TRN FP8_EXP4: max finite 240, inf beyond


def _f8(x):
    return np.clip(x, -240.0, 240.0).astype(F8NP)


def build_program(C=N_CORES, B=B_BANDS, D=D_MODEL):
    import concourse.bass as bass
    import concourse.mybir as mybir
    from concourse import bacc, tile
    from concourse.masks import make_identity

    F32 = mybir.dt.float32
    F16 = mybir.dt.float16
    F8 = mybir.dt.float8e4
    DR = mybir.MatmulPerfMode.DoubleRow

    N = C * B * QT                 # total context
    NQ = B * QT                    # rows per core
    DP = D // 128                  # d tiles
    DPP = DP // 2                  # d-tile pairs (DR fp8 packs 2 per MM)
    JW = min(512, QT * C)          # j / free-dim window
    NW = N // JW                   # S windows over full context
    NJT = N // 128                 # j tiles
    MASKW = QT * C                 # mask window width (last cols of each extent)
    NDC = D // JW                  # output d2 chunks

    exts = [QT * C * (B - t) for t in range(B)]   # extent per local q-tile t

    def n_jt(jt):                  # active moving width at j-tile jt
        return 128 * (B - jt // C)

    nc = bacc.Bacc("TRN2", target_bir_lowering=False, debug=False)

    # inputs (pre-tiled on host for contiguous DMA)
    qrth_d = nc.dram_tensor("qrth", [128, DP, NQ], F16, kind="ExternalInput")
    qrt8_d = nc.dram_tensor("qrt8", [128, DPP, 2, NQ], F8, kind="ExternalInput")
    # wq tensors come in PAIRS of output d-tiles (dim "two"): one DMA trigger
    # loads two d-tiles' weights, halving trigger count / semaphore pressure
    wqh_d = nc.dram_tensor("wqh", [DP // 2, 128, 2, DP, 128], F16,
                           kind="ExternalInput")
    wq8_d = nc.dram_tensor("wq8", [DP // 2, 128, 2, DPP, 2, 128], F8,
                           kind="ExternalInput")
    eth_d = nc.dram_tensor("eth", [NW, 128, DP, JW], F16, kind="ExternalInput")
    e8_d = nc.dram_tensor("e8", [NW, 128, DPP, 2, JW], F8, kind="ExternalInput")
    ev_d = nc.dram_tensor("ev", [DP, 128, NJT, 128], F16, kind="ExternalInput")
    ov_d = nc.dram_tensor("ov", [NDC, 128, DP, JW], F16, kind="ExternalInput")
    mask_d = nc.dram_tensor("mask", [128, MASKW], F32, kind="ExternalInput")
    out_d = nc.dram_tensor("out", [NQ, D], F32, kind="ExternalOutput")

    with tile.TileContext(nc) as tc:
        with (
            tc.tile_pool(name="const", bufs=1) as constp,
            tc.tile_pool(name="qt", bufs=1) as qtp,
            tc.tile_pool(name="pt", bufs=1) as ptp,
            tc.tile_pool(name="small", bufs=1) as smallp,
            tc.tile_pool(name="mm_ps", bufs=4, space="PSUM") as mmps,
            tc.tile_pool(name="tr_ps", bufs=2, space="PSUM") as trps,
            tc.tile_pool(name="pv_ps", bufs=2, space="PSUM") as pvps,
        ):
            ident = constp.tile([128, 128], F16, tag="ident")
            make_identity(nc, ident[:])
            # mask load is issued later (after the Q-phase prologue DMAs) to
            # keep the first matmul's deps at the head of the DMA queues
            mask_sb = constp.tile([128, MASKW], F32, tag="mask")

            # PT[jt]: transposed attention weights, [j-part, q-cols prefix]
            pt = [ptp.tile([128, n_jt(jt)], F16, tag=f"pt{jt}", name=f"pt{jt}") for jt in range(NJT)]

            # S-phase stationary: qth = fp16(256*Q^T); q8[a] packs the fp8
            # low parts (ql256) of d-tiles 2a and 2a+1 as its two DR planes
            qth = [qtp.tile([128, NQ], F16, tag=f"qth{dp}", name=f"qth{dp}") for dp in range(DP)]
            q8 = [qtp.tile([128, 2, NQ], F8, tag=f"q8_{a}", name=f"q8_{a}") for a in range(DPP)]

            negmax = [smallp.tile([128, 1], F32, tag=f"ngm{t}", name=f"ngm{t}") for t in range(B)]
            rsum = [smallp.tile([128, 1], F32, tag=f"rs{t}", name=f"rs{t}") for t in range(B)]
            recip = [smallp.tile([128, 1], F32, tag=f"rc{t}", name=f"rc{t}") for t in range(B)]
            rspart = {}
            rmax = [smallp.tile([128, NW], F32, tag=f"rmx{t}", name=f"rmx{t}")
                    for t in range(B)]

            # ev pool opens before ew (proper nesting: released after PV)
            # so value tiles can prefetch during the S tail on the scalar
            # DMA queue, not head-blocking ew loads
            evp_cm = tc.tile_pool(name="evs", bufs=5)
            evp = evp_cm.__enter__()
            ev_tiles = {}

            def load_ev(dt):
                evs = evp.tile([128, NJT, 128], F16, tag="evs", name="evs")
                nc.scalar.dma_start(evs[:], ev_d[dt])
                ev_tiles[dt] = evs

            # ew pool lives across Q+S so window 0 can prefetch during Q
            ewp_cm = tc.tile_pool(name="ew", bufs=2)
            ewp = ewp_cm.__enter__()
            ew_tiles = {}

            def load_window(w):
                ewh = ewp.tile([128, DP, JW], F16, tag="ewh", name="ewh")
                ew8 = ewp.tile([128, DPP, 2, JW], F8, tag="ew8", name="ew8")
                nc.sync.dma_start(ewh[:], eth_d[w])
                nc.sync.dma_start(ew8[:], e8_d[w])
                ew_tiles[w] = (ewh, ew8)

            # ---------------- Phase Q: 256*Q^T = (256*qk)^T-contracted rows
            with (
                tc.tile_pool(name="qrt", bufs=1) as qrtp,
                tc.tile_pool(name="wq", bufs=3) as wqp,
                tc.tile_pool(name="qtmp", bufs=2) as qtmpp,
            ):
                wq_tiles = {}

                def load_wq2(g):
                    # loads output d-tiles 2g and 2g+1 in two triggers
                    wqh_sl = wqp.tile([128, 2, DP, 128], F16, tag="wqh", name="wqh")
                    wq8_sl = wqp.tile([128, 2, DPP, 2, 128], F8, tag="wq8", name="wq8")
                    nc.sync.dma_start(wqh_sl[:], wqh_d[g])
                    nc.sync.dma_start(wq8_sl[:], wq8_d[g])
                    wq_tiles[g] = (wqh_sl, wq8_sl)

                qrt_h = qrtp.tile([128, DP, NQ], F16, tag="qrh", name="qrh")
                qrt_8 = qrtp.tile([128, DPP, 2, NQ], F8, tag="qr8", name="qr8")
                # DMA triggers cost ~0.7us each on the sync engine and the
                # first ~6 get dedicated semaphores, so the prologue uses few,
                # fat, partition-major transfers ordered by first use: wq0's
                # first half + early qrt chunks gate the first matmuls.

                # stage the first matmuls' deps so the PE starts early
                wqh_sl0 = wqp.tile([128, 2, DP, 128], F16, tag="wqh", name="wqh")
                wq8_sl0 = wqp.tile([128, 2, DPP, 2, 128], F8, tag="wq8", name="wq8")
                wq_tiles[0] = (wqh_sl0, wq8_sl0)
                # qrt stream on the (idle) scalar engine's DMA queue, wq
                # stream on sync: triggers issue in parallel, no FIFO
                # head-blocking between the two streams
                nc.sync.dma_start(wqh_sl0[:, 0, 0:DP // 2, :],
                                  wqh_d[0][:, 0, 0:DP // 2, :])
                nc.scalar.dma_start(qrt_h[:, 0:4, :], qrth_d[:, 0:4, :])
                nc.scalar.dma_start(qrt_h[:, 4:8, :], qrth_d[:, 4:8, :])
                nc.sync.dma_start(wqh_sl0[:, 0, DP // 2:, :],
                                  wqh_d[0][:, 0, DP // 2:, :])
                nc.scalar.dma_start(qrt_h[:, 8:12, :], qrth_d[:, 8:12, :])
                nc.scalar.dma_start(qrt_h[:, 12:DP, :], qrth_d[:, 12:DP, :])
                nc.sync.dma_start(wq8_sl0[:, 0], wq8_d[0][:, 0])
                nc.scalar.dma_start(qrt_8[:, 0:DPP // 2], qrt8_d[:, 0:DPP // 2])
                nc.scalar.dma_start(qrt_8[:, DPP // 2:], qrt8_d[:, DPP // 2:])
                nc.sync.dma_start(wqh_sl0[:, 1], wqh_d[0][:, 1])
                nc.sync.dma_start(wq8_sl0[:, 1], wq8_d[0][:, 1])
                load_wq2(1)
                load_wq2(2)
                nc.sync.dma_start(mask_sb[:], mask_d[:])

                for dp in range(DP):
                    wqh_sl, wq8_sl = wq_tiles[dp // 2]
                    h = dp % 2
                    ps = mmps.tile([128, NQ], F32, tag="mm")
                    for dk in range(DP):
                        nc.tensor.matmul(ps[:], wqh_sl[:, h, dk], qrt_h[:, dk, :],
                                         start=(dk == 0), stop=False)
                    for a in range(DPP):
                        nc.tensor.matmul(ps[:], wq8_sl[:, h, a], qrt_8[:, a],
                                         start=False, stop=(a == DPP - 1),
                                         perf_mode=DR)
                    if h == 1:
                        wq_tiles.pop(dp // 2)
                    if h == 0 and dp // 2 + 3 < DP // 2:
                        load_wq2(dp // 2 + 3)
                    if dp == 2:
                        load_window(0)
                    nc.vector.tensor_copy(qth[dp][:], ps[:])
                    qh32 = qtmpp.tile([128, NQ], F32, tag="qh32")
                    nc.vector.tensor_copy(qh32[:], qth[dp][:])
                    # low fp8 plane: 256*Q - fp16(256*Q), already at 2^8 scale
                    nc.vector.tensor_sub(q8[dp // 2][:, dp % 2, :], ps[:], qh32[:])

            # ---------------- Phase S: scores + softmax + P^T
            with (
                tc.tile_pool(name="s", bufs=1) as sp,
                tc.tile_pool(name="p", bufs=2) as pp,
            ):
                s_t = [sp.tile([128, exts[t]], F32, tag=f"s{t}", name=f"s{t}") for t in range(B)]

                def softmax_t(t):
                    ext = exts[t]
                    nc.vector.reduce_max(
                        out=negmax[t][:], in_=rmax[t][:, :ext // JW],
                        axis=mybir.AxisListType.X, negate=True)
                    # descale: bias for exp must be -rowmax = -rowmax256/256
                    nc.vector.tensor_scalar_mul(
                        negmax[t][:], negmax[t][:], 1.0 / LSCALE)
                    for w2 in range(ext // JW):
                        pwin = pp.tile([128, JW], F16, tag=f"p{t}", name=f"p{t}")
                        rp = smallp.tile([128, 1], F32, tag=f"rsp{t}_{w2}", name=f"rsp{t}_{w2}")
                        rspart[(t, w2)] = rp
                        nc.scalar.activation(
                            pwin[:], s_t[t][:, w2 * JW:(w2 + 1) * JW],
                            mybir.ActivationFunctionType.Exp,
                            bias=negmax[t][:], scale=1.0 / LSCALE, accum_out=rp[:])
                        for jj in range(JW // 128):
                            jt = w2 * (JW // 128) + jj
                            trp = trps.tile([128, 128], F16, tag="tr")
                            nc.tensor.transpose(
                                trp[:], pwin[:, jj * 128:(jj + 1) * 128], ident[:])
                            nc.vector.tensor_copy(
                                pt[jt][:, t * 128:(t + 1) * 128], trp[:])
                    # rowsum = sum of window partials; recip
                    nc.vector.tensor_copy(rsum[t][:], rspart[(t, 0)][:])
                    for w2 in range(1, ext // JW):
                        nc.vector.tensor_add(
                            rsum[t][:], rsum[t][:], rspart[(t, w2)][:])
                    nc.vector.reciprocal(recip[t][:], rsum[t][:])

                for w in range(NW):
                    if w + 1 < NW:
                        load_window(w + 1)
                    if w == NW - 2:
                        for _g in range(3):
                            load_ev(_g)
                    elif w == NW - 1:
                        load_ev(3)
                        load_ev(4)
                    ewh, ew8 = ew_tiles.pop(w)
                    for t in range(B):
                        if exts[t] <= JW * w:
                            continue
                        ps = mmps.tile([128, JW], F32, tag="mm")
                        for dp in range(DP):
                            nc.tensor.matmul(ps[:], qth[dp][:, t * 128:(t + 1) * 128],
                                             ewh[:, dp],
                                             start=(dp == 0), stop=False)
                        for a in range(DPP):
                            nc.tensor.matmul(ps[:], q8[a][:, :, t * 128:(t + 1) * 128],
                                             ew8[:, a],
                                             start=False, stop=(a == DPP - 1),
                                             perf_mode=DR)
                        # copy scores to SBUF, folding in the causal mask on
                        # the last MASKW columns; track per-window row max
                        nmw = MASKW // JW
                        wloc = exts[t] // JW - 1 - w   # windows from the end
                        if wloc < nmw:
                            moff = (nmw - 1 - wloc) * JW
                            nc.vector.tensor_add(
                                s_t[t][:, w * JW:(w + 1) * JW], ps[:],
                                mask_sb[:, moff:moff + JW])
                        else:
                            nc.vector.tensor_copy(
                                s_t[t][:, w * JW:(w + 1) * JW], ps[:])
                        nc.vector.reduce_max(
                            out=rmax[t][:, w:w + 1],
                            in_=s_t[t][:, w * JW:(w + 1) * JW],
                            axis=mybir.AxisListType.X)
                        if JW * (w + 1) == exts[t]:
                            softmax_t(t)

            ewp_cm.__exit__(None, None, None)

            # ---------------- Phase PV: OpT[d, q] = sum_j E[j,d] P^T[j,q]
            with (
                tc.tile_pool(name="opt", bufs=1) as optp,
                tc.tile_pool(name="ovs", bufs=2) as ovp,
                tc.tile_pool(name="osb", bufs=2) as osbp,
            ):
                ov_tiles = {}

                def load_ov(dc):
                    ovs = ovp.tile([128, DP, JW], F16, tag="ovs", name="ovs")
                    nc.scalar.dma_start(ovs[:], ov_d[dc])
                    ov_tiles[dc] = ovs

                opt = [optp.tile([128, NQ], F16, tag=f"opt{dt}", name=f"opt{dt}") for dt in range(DP)]
                NPRE = min(5, DP)   # tiles 0..4 already prefetched in S tail
                for dt in range(DP):
                    evs = ev_tiles.pop(dt)
                    ps = pvps.tile([128, NQ], F32, tag="pv")
                    for jt in range(NJT):
                        nw_ = n_jt(jt)
                        nc.tensor.matmul(ps[:, :nw_], evs[:, jt], pt[jt][:, :nw_],
                                         start=(jt == 0), stop=(jt == NJT - 1))
                    if dt + NPRE < DP:
                        load_ev(dt + NPRE)
                    elif dt == max(0, DP - NPRE):
                        load_ov(0)
                    elif dt == max(1, DP - NPRE + 1):
                        load_ov(1)
                    nc.vector.tensor_copy(opt[dt][:], ps[:])

                # ---------------- Phase O: out = (OpT^T @ ov) * recip
                if True:
                    for dc in range(NDC):
                        if dc + 2 < NDC:
                            load_ov(dc + 2)
                        ovs = ov_tiles.pop(dc)
                        for t in range(B):
                            ps = mmps.tile([128, JW], F32, tag="mm")
                            for dt in range(DP):
                                nc.tensor.matmul(
                                    ps[:], opt[dt][:, t * 128:(t + 1) * 128],
                                    ovs[:, dt],
                                    start=(dt == 0), stop=(dt == DP - 1))
                            osb = osbp.tile([128, JW], F32, tag="osb")
                            nc.vector.tensor_scalar_mul(osb[:], ps[:], recip[t][:])
                            nc.sync.dma_start(
                                out_d[t * 128:(t + 1) * 128,
                                      dc * JW:(dc + 1) * JW], osb[:])

            evp_cm.__exit__(None, None, None)

    nc.compile()
    return nc


def make_in_maps(embedding, qk, ov, C=N_CORES, B=B_BANDS):
    """Host-side layout/dtype prep. Returns (in_maps, gtiles_per_core)."""
    N, D = embedding.shape
    DP = D // 128
    DPP = DP // 2
    JW = min(512, QT * C)
    NW = N // JW
    NJT = N // 128
    NQ = B * QT
    NDC = D // JW
    MASKW = QT * C

    E = np.ascontiguousarray(embedding.astype(np.float32))
    ET = np.ascontiguousarray(E.T)
    Eh = E.astype(np.float16)
    ETh = np.ascontiguousarray(ET.astype(np.float16))
    ETh8 = _f8(ETh.astype(np.float32))
    W = qk.astype(np.float32)
    Wh = W.astype(np.float16)
    Wl = W - Wh.astype(np.float32)
    # 256*Wh is exact in fp16 (|W| < 1 so |256*Wh| < 65504)
    Wh256 = (LSCALE * Wh.astype(np.float32)).astype(np.float16)
    Wl8 = _f8(LSCALE * Wl)
    OVh = ov.astype(np.float16)

    eth_t = np.ascontiguousarray(
        ETh.reshape(DP, 128, NW, JW).transpose(2, 1, 0, 3))
    # e8[w, b, a, i, j] = fp8(ETh)[(2a+i)*128+b, w*JW+j]  (DR d-tile pairs)
    e8_t = np.ascontiguousarray(
        ETh8.reshape(DPP, 2, 128, NW, JW).transpose(3, 2, 0, 1, 4))
    # paired layout: wqh[c2, b, j, a, d] = 256*Wh[a*128+b, (2*c2+j)*128+d]
    wqh_t = np.ascontiguousarray(
        Wh256.reshape(DP, 128, DPP, 2, 128).transpose(2, 1, 3, 0, 4))
    # wq8[c2, b, j, a, i, d] = fp8(256*Wl)[(2a+i)*128+b, (2*c2+j)*128+d]
    wq8_t = np.ascontiguousarray(
        Wl8.reshape(DPP, 2, 128, DPP, 2, 128).transpose(3, 2, 4, 0, 1, 5))
    ev_t = np.ascontiguousarray(
        Eh.reshape(NJT, 128, DP, 128).transpose(2, 1, 0, 3))
    ov_t = np.ascontiguousarray(
        OVh.reshape(DP, 128, NDC, JW).transpose(2, 1, 0, 3))

    r = np.arange(128)[:, None]
    m = np.arange(MASKW)[None, :]

    in_maps = []
    gtiles_all = []
    for i in range(C):
        gtiles = [C * (B - 1 - t) + i for t in range(B)]
        gtiles_all.append(gtiles)
        qrh = np.concatenate(
            [ETh[:, 128 * g:128 * (g + 1)] for g in gtiles], axis=1)
        qr8h = np.concatenate(
            [ETh8[:, 128 * g:128 * (g + 1)] for g in gtiles], axis=1)
        # partition-major for fat contiguous DMA runs:
        # qrth[b, dk, q] = ETh_own[dk*128+b, q]
        # qrt8[b, a, i, q] = fp8(ETh_own)[(2a+i)*128+b, q]
        qrth = qrh.reshape(DP, 128, NQ).transpose(1, 0, 2)
        qrt8 = qr8h.reshape(DPP, 2, 128, NQ).transpose(2, 0, 1, 3)
        mask = np.where(m <= 128 * i + r, 0.0, MASK_NEG).astype(np.float32)
        in_maps.append({
            "qrth": np.ascontiguousarray(qrth),
            "qrt8": np.ascontiguousarray(qrt8),
            "wqh": wqh_t, "wq8": wq8_t,
            "eth": eth_t, "e8": e8_t,
            "ev": ev_t, "ov": ov_t,
            "mask": mask,
        })
    return in_maps, gtiles_all


_CACHED = {}


def kernel(embedding, qk, ov):
    from concourse.bass_utils import run_bass_kernel_spmd

    key = "main"
    if key not in _CACHED:
        _CACHED[key] = build_program()
    nc = _CACHED[key]

    in_maps, gtiles_all = make_in_maps(embedding, qk, ov)
    res = run_bass_kernel_spmd(nc, in_maps, core_ids=list(range(N_CORES)))

    N, D = embedding.shape
    out = np.empty((N, D), dtype=np.float32)
    for i in range(N_CORES):
        o = res.results[i]["out"]
        for t, g in enumerate(gtiles_all[i]):
            out[128 * g:128 * (g + 1)] = o[128 * t:128 * (t + 1)]
    return out
